# revision 10
# baseline (speedup 1.0000x reference)
"""Trainium2 Bass kernel for suffix-softmax attention visualization.

Computes, for hidden_states [S, B, H], W [H, 1], b [1]:
    s[t, b]   = sum_h hidden_states[t, b, h] * W[h, 0] + b[0]
    out[t, b] = exp(s[t, b]) / sum_{t' >= t} exp(s[t', b])     (suffix softmax)
returned as [S, B, 1] f32.

The softmax ratio is shift-invariant, so the scalar bias b cancels exactly
and is not needed on device. Scores are N(0,1)-scaled by construction, so
exp() needs no max-subtraction.

Sharding: data-parallel over the batch axis — 8 NeuronCores, 8 batch
columns each; 64 MB of f32 per core, a pure HBM stream.

v3 design notes (what was measured on the way here):
  - The obvious SWDGE cast-DMA input stream (f32 HBM -> fp16 SBUF) hits
    the known SDMA-engine-15 slowdown: SWDGE descriptor rings contend
    with engine 15's SBUF AXI port, making it ~21% slower than the other
    15 engines. Since each engine owns 8 fixed partitions, every block
    waits on engine 15 and the stream ends in a ~35 us single-engine
    tail (measured: E15 199 us busy vs 168 us for the rest).
  - HWDGE (sync/scalar rings) has no descriptor ring and is immune —
    measured perfectly uniform 718 ns / 16 KB-read packets on all 16
    engines. But HWDGE cannot cast, and f32 compute on DVE/Pool runs
    ~2.4x slower per element (two-input f32 ops are read-bound), so an
    all-f32 pipeline is compute-bound (~310 us).
  - Hence v3: stream RAW f32 over the Sync HWDGE ring (uniform ~5.75 us
    per [128 seq, 8 b, 512 h] block), and cast each block to fp16 on the
    otherwise-idle Pool engine (tensor_copy, ~0.9 us per 2-column chunk,
    ~3.6 us/block). Compute then runs the proven fp16 pipeline:
      DVE : one 2x_1p tensor_tensor multiply for the ACT columns
            (~1.1 us) + 4 fused STT columns (~0.77 us each) + recip
      ACT : 4 copy-accumulate h-reductions (~1.0 us each) + exp
      Pool: the cast chunks + the finalize multiply
      PE  : tri/triu matmuls for the suffix state
    Every engine sits at <= ~4.7 us/block against the ~5.75 us DMA slot.
  - blocks stream in REVERSE seq order (suffix accumulates forward);
    the suffix state lives in one PSUM tile R [128, 8]:
    matmul-accumulating lower-triangular ones gives R + within-block
    suffix-scan (the divisor), then strictly-upper ones turn it into the
    next running total, broadcast across partitions, on the idle PE;
  - the finalize (reciprocal on DVE, multiply on Pool) is deferred one
    block so nothing waits on the exp -> matmul chain;
  - outputs collect in SBUF and DMA out in 4-block chunks on the Scalar
    HWDGE ring (separate FIFO from the input stream).
"""

import numpy as np

import concourse.bacc as bacc
import concourse.mybir as mybir
import concourse.tile as tile
from concourse import bass_utils

P = 128
S = 4096
B = 64
H = 512
N_CORES = 8
BC = B // N_CORES  # batch columns per core
NBLK = S // P


def build_program(
    hs32_bufs=6, hs16_bufs=6, out_chunk=4, look=4, act_cols=4,
    act_prod="dve_tt", cast_chunk=2, Bc=BC
):
    """Build the per-core Bass program.

    Inputs : hs [S, Bc, H] f32, wb [128, H] fp16 (W broadcast),
             wbr [128, act_cols*H] fp16 (W broadcast, repeated),
             tri [128, 128] f32 lower-triangular ones (suffix scan),
             triu [128, 128] f32 strictly-upper ones (running-total update).
    Output : out [S, Bc] f32.

    act_cols : columns h-reduced by ACT copy-accumulate (products
               materialized per `act_prod`); the rest are fused STT on DVE.
    act_prod : "dve_tt"  — fp16 2x tensor_tensor on DVE (after the cast);
               "pool_tt" — f32 tensor_tensor (cast on write) on Pool,
                           skipping the cast for those columns.
    cast_chunk: columns per Pool cast instruction.
    """
    assert S % P == 0
    nblk = S // P
    assert nblk % out_chunk == 0
    assert hs32_bufs >= look + 2

    nc = bacc.Bacc("TRN2", target_bir_lowering=False, debug=False)
    hs = nc.dram_tensor("hs", [S, Bc, H], mybir.dt.float32, kind="ExternalInput")
    wb = nc.dram_tensor("wb", [P, H], mybir.dt.float16, kind="ExternalInput")
    wbr = nc.dram_tensor(
        "wbr", [P, max(act_cols, 1) * H], mybir.dt.float16, kind="ExternalInput"
    )
    wbr32 = nc.dram_tensor(
        "wbr32", [P, max(act_cols, 1) * H], mybir.dt.float32, kind="ExternalInput"
    )
    tri = nc.dram_tensor("tri", [P, P], mybir.dt.float32, kind="ExternalInput")
    triu = nc.dram_tensor("triu", [P, P], mybir.dt.float32, kind="ExternalInput")
    out = nc.dram_tensor("out", [S, Bc], mybir.dt.float32, kind="ExternalOutput")

    # Processing order: last seq block first (suffix accumulates forward).
    order = list(range(nblk - 1, -1, -1))
    # cols-per-DMA-chunk by processing index: small chunks at the ends so
    # compute starts early (ramp) and the drain overlaps per-column.
    split_plan = {0: 2, 1: 4, nblk - 1: 2}

    dve_lo = act_cols  # columns [act_cols, Bc) are fused STT on DVE

    with tile.TileContext(nc) as tc:
        with (
            tc.tile_pool(name="h32p", bufs=hs32_bufs) as h32p,
            tc.tile_pool(name="h16p", bufs=hs16_bufs) as h16p,
            tc.tile_pool(name="consts", bufs=1) as consts,
            tc.tile_pool(name="work", bufs=1) as work,
            tc.tile_pool(name="sp", bufs=4) as sp,
            tc.tile_pool(name="ep", bufs=4) as ep,
            tc.tile_pool(name="lsep", bufs=3) as lsep,
            tc.tile_pool(name="prodp", bufs=3) as prodp,
            tc.tile_pool(name="psum", bufs=1, space="PSUM") as psum,
        ):
            hs_ap = hs.ap()
            hs32_tiles = {}
            hs16_tiles = {}

            def issue_dma(idx):
                j = order[idx]
                hst = h32p.tile([P, Bc, H], mybir.dt.float32)
                rows = hs_ap[j * P : (j + 1) * P, :, :]
                qb = min(split_plan.get(idx, Bc), Bc)
                for q in range(0, Bc, qb):
                    nc.sync.dma_start(
                        out=hst[:, q : q + qb, :], in_=rows[:, q : q + qb, :]
                    )
                hs32_tiles[j] = hst

            wb_t = consts.tile([P, H], mybir.dt.float16)
            nc.scalar.dma_start(out=wb_t, in_=wb.ap())
            if act_cols and act_prod == "dve_tt":
                wbr_t = consts.tile([P, act_cols * H], mybir.dt.float16)
                nc.scalar.dma_start(out=wbr_t, in_=wbr.ap()[:, : act_cols * H])
            if act_cols and act_prod == "pool_tt":
                wbr32_t = consts.tile([P, act_cols * H], mybir.dt.float32)
                nc.scalar.dma_start(out=wbr32_t, in_=wbr32.ap()[:, : act_cols * H])
            tri_t = consts.tile([P, P], mybir.dt.float32)
            nc.scalar.dma_start(out=tri_t, in_=tri.ap())
            triu_t = consts.tile([P, P], mybir.dt.float32)
            nc.scalar.dma_start(out=triu_t, in_=triu.ap())

            for idx in range(look):
                issue_dma(idx)

            # Separate per-engine throwaway out-tiles: sharing one creates
            # false WAW dependencies that serialize the engines.
            dummy_v = work.tile([P, H], mybir.dt.float16)
            dummy_act = work.tile([P, H], mybir.dt.float16)
            sel_buf = work.tile([P, nblk * Bc], mybir.dt.float32)
            r_ps = psum.tile([P, Bc], mybir.dt.float32)

            out_ap = out.ap().rearrange("(blk p) b -> p blk b", p=P)

            def emit_cast(j):
                """Pool: cast the f32 block to fp16, chunked by columns.
                Columns handled by pool_tt products skip the cast."""
                hst = hs32_tiles[j]
                h16 = h16p.tile([P, Bc, H], mybir.dt.float16)
                lo = 0 if act_prod == "dve_tt" else act_cols
                for q in range(lo, Bc, cast_chunk):
                    qe = min(q + cast_chunk, Bc)
                    nc.gpsimd.tensor_copy(
                        out=h16[:, q:qe, :].rearrange("p b h -> p (b h)"),
                        in_=hst[:, q:qe, :].rearrange("p b h -> p (b h)"),
                    )
                hs16_tiles[j] = h16

            def emit_finalize(j, s_t, e_t):
                lo = j * Bc
                # sel = e * (1/(R + scan)): reciprocal on DVE (ACT's is
                # banned for accuracy), multiply on Pool.
                rec_t = lsep.tile([P, Bc], mybir.dt.float32)
                nc.vector.reciprocal(rec_t, r_ps)
                nc.gpsimd.tensor_mul(sel_buf[:, lo : lo + Bc], e_t, rec_t)
                if j == 1:
                    # Flush blocks 1..out_chunk-1 early so the very last DMA
                    # (after block 0's finalize) is a single small block.
                    sel_ap = sel_buf[:, Bc : out_chunk * Bc].rearrange(
                        "p (blk b) -> p blk b", b=Bc
                    )
                    nc.scalar.dma_start(out=out_ap[:, 1:out_chunk, :], in_=sel_ap)
                elif j == 0:
                    sel_ap = sel_buf[:, 0:Bc].rearrange(
                        "p (blk b) -> p blk b", b=Bc
                    )
                    nc.scalar.dma_start(out=out_ap[:, 0:1, :], in_=sel_ap)
                elif j % out_chunk == 0:
                    sel_ap = sel_buf[:, lo : lo + out_chunk * Bc].rearrange(
                        "p (blk b) -> p blk b", b=Bc
                    )
                    nc.scalar.dma_start(
                        out=out_ap[:, j : j + out_chunk, :], in_=sel_ap
                    )

            emit_cast(order[0])

            pending = None  # (j, s_t, e_t) awaiting its deferred finalize
            for idx, j in enumerate(order):
                hst32 = hs32_tiles[j]
                h16 = hs16_tiles[j]
                s_t = sp.tile([P, Bc], mybir.dt.float32)
                e_t = ep.tile([P, Bc], mybir.dt.float32)

                # Cast the NEXT block right behind this block's DMA so Pool
                # stays one block ahead of the consumers.
                if idx + 1 < nblk:
                    emit_cast(order[idx + 1])

                if act_cols:
                    prod_t = prodp.tile([P, act_cols, H], mybir.dt.float16)
                    if act_prod == "dve_tt":
                        nc.vector.tensor_tensor(
                            prod_t.rearrange("p b h -> p (b h)"),
                            h16[:, :act_cols, :].rearrange("p b h -> p (b h)"),
                            wbr_t,
                            op=mybir.AluOpType.mult,
                        )
                    else:
                        nc.gpsimd.tensor_tensor(
                            prod_t.rearrange("p b h -> p (b h)"),
                            hst32[:, :act_cols, :].rearrange("p b h -> p (b h)"),
                            wbr32_t,
                            op=mybir.AluOpType.mult,
                        )
                    for b in range(act_cols):
                        nc.scalar.activation(
                            dummy_act,
                            prod_t[:, b, :],
                            mybir.ActivationFunctionType.Copy,
                            accum_out=s_t[:, b : b + 1],
                        )
                for b in range(dve_lo, Bc):
                    nc.vector.scalar_tensor_tensor(
                        out=dummy_v,
                        in0=h16[:, b, :],
                        scalar=1.0,
                        in1=wb_t,
                        op0=mybir.AluOpType.mult,
                        op1=mybir.AluOpType.mult,
                        accum_out=s_t[:, b : b + 1],
                    )

                # Deferred finalize of the previous block: its R+scan divisor
                # has been sitting ready in PSUM, so nothing waits on the
                # cross-engine chain.
                if pending is not None:
                    pj, ps, pe = pending
                    emit_finalize(pj, ps, pe)
                    # R <- R + total(prev block), broadcast on all partitions.
                    # Must run after the reciprocal's read of R.
                    nc.tensor.matmul(r_ps, triu_t, pe, start=False, stop=True)

                if idx + look < nblk:
                    issue_dma(idx + look)

                nc.scalar.activation(
                    e_t, s_t, mybir.ActivationFunctionType.Exp
                )
                # R + within-block suffix scan -> the divisor for block j.
                nc.tensor.matmul(r_ps, tri_t, e_t, start=(idx == 0), stop=True)
                pending = (j, s_t, e_t)

            pj, ps, pe = pending
            emit_finalize(pj, ps, pe)

    nc.compile()
    return nc


_PROGRAM = None


def _get_program():
    global _PROGRAM
    if _PROGRAM is None:
        _PROGRAM = build_program()
    return _PROGRAM


def make_in_maps(hidden_states, W, act_cols=4):
    hidden_states = np.asarray(hidden_states, dtype=np.float32)
    W = np.asarray(W, dtype=np.float32)
    wrow = np.broadcast_to(W[:, 0][None, :], (P, H))
    wb = np.ascontiguousarray(wrow.astype(np.float16))
    wbr = np.ascontiguousarray(np.tile(wb, (1, max(act_cols, 1))))
    wbr32 = np.ascontiguousarray(
        np.tile(wrow.astype(np.float32), (1, max(act_cols, 1)))
    )
    tri = np.tril(np.ones((P, P), dtype=np.float32))
    triu = np.triu(np.ones((P, P), dtype=np.float32), 1)
    in_maps = []
    for c in range(N_CORES):
        hs_c = np.ascontiguousarray(hidden_states[:, c * BC : (c + 1) * BC, :])
        in_maps.append(
            {"hs": hs_c, "wb": wb, "wbr": wbr, "wbr32": wbr32,
             "tri": tri, "triu": triu}
        )
    return in_maps


def assemble_output(results):
    cols = [results[c]["out"] for c in range(N_CORES)]
    return np.concatenate(cols, axis=1)[..., None].astype(np.float32)


def kernel(hidden_states, W, b):
    nc = _get_program()
    in_maps = make_in_maps(hidden_states, W)
    res = bass_utils.run_bass_kernel_spmd(nc, in_maps, core_ids=list(range(N_CORES)))
    return assemble_output(res.results)


# revision 14
# speedup vs baseline: 2.5961x; 2.5961x over previous
"""Trainium2 Bass kernel for suffix-softmax attention visualization.

Computes, for hidden_states [S, B, H], W [H, 1], b [1]:
    s[t, b]   = sum_h hidden_states[t, b, h] * W[h, 0] + b[0]
    out[t, b] = exp(s[t, b]) / sum_{t' >= t} exp(s[t', b])     (suffix softmax)
returned as [S, B, 1] f32.

The softmax ratio is shift-invariant, so the scalar bias b cancels exactly
and is not needed on device. Scores are N(0,1)-scaled by construction, so
exp() needs no max-subtraction.

Sharding: data-parallel over the batch axis — 8 NeuronCores, 8 batch
columns each; 64 MB of f32 per core, a pure HBM stream.

Architecture (v4, superblock). Measured context:
  - The input must land in SBUF as fp16 via the SWDGE cast-DMA: raw-f32
    landings double SBUF port traffic past the ~7 KB/us/partition wall
    (engines degrade 2.5-4x), and only SWDGE can cast.
  - Under SWDGE, SDMA engine 15 suffers periodic stall clusters (~every
    13 us, +0.5 us each) making it ~21% slower than the other 15
    engines; engines own fixed partition sets, so the whole stream ends
    up paced by engine 15 (known TRN2 engine-7/15 SWDGE issue).
  - This kernel streams in 512-seq-row SUPERBLOCKS: partition p holds
    the four consecutive rows t = 512k + 4p + i. Per partition that is
    one contiguous 64 KB f32 read -> 32 KB fp16 write descriptor — 4x
    fewer descriptors and semaphore packets than row-per-partition
    blocks (attacking the stall-cluster frequency) and near-asymptotic
    per-packet efficiency.
  - Per supertile (4096 scores): DVE runs one 2x_1p tensor_tensor
    multiply producing products for the 16 ACT-reduced (row, col) pairs
    (~4.5 us) plus 16 fused STT columns (~0.77 us each); ACT
    copy-accumulates the 16 product columns (~1.0 us each) + one exp;
    Pool issues the cast-DMAs and does the small suffix adds and the
    finalize multiply. Every engine stays <= ~85% of the ~20.5 us
    supertile DMA slot.
  - Suffix-softmax state: R [128, Bc] in PSUM holds the running total
    of exp over all later supertiles, broadcast across partitions.
    Per supertile: T[p] = sum_i e_i[p] (3 small adds); matmul with
    strictly-lower ones gives U[m] = R + sum_{p>m} T[p]; the divisors
    are the within-partition suffix d_i = e_i + d_{i+1}, d_3 = e_3 + U
    (4 small adds); a second matmul with upper-inclusive ones advances
    R' = U + sum_{p<=m} T = R + sum T, broadcast. The finalize
    (suffix adds, reciprocal on DVE, multiply on Pool) is deferred one
    supertile so nothing waits on the exp -> matmul chain.
  - outputs collect in SBUF and DMA out per supertile on the Sync ring.
"""

import numpy as np

import concourse.bacc as bacc
import concourse.mybir as mybir
import concourse.tile as tile
from concourse import bass_utils

P = 128
S = 4096
B = 64
H = 512
N_CORES = 8
BC = B // N_CORES  # batch columns per core
R4 = 4             # seq rows per partition per supertile
SB = P * R4        # seq rows per supertile
NSUP = S // SB     # number of supertiles


def build_program(bufs16=3, look=2, act_cols=4, prod_on="dve", Bc=BC):
    """Build the per-core Bass program.

    Inputs : hs [S, Bc, H] f32,
             wb [128, H] fp16 (W broadcast across partitions),
             wbr [128, R4*act_cols*H] fp16 (W tiled for the product TT),
             trilS [128, 128] f32 ones strictly-lower (p > m),
             triuI [128, 128] f32 ones upper-inclusive (p <= m).
    Output : out [S, Bc] f32.

    act_cols: batch columns per row h-reduced by ACT copy-accumulate
    (products via one tensor_tensor on `prod_on`); the remaining
    columns are fused STT on DVE.
    """
    nsup = NSUP
    nw = R4 * Bc  # scores per partition per supertile

    nc = bacc.Bacc("TRN2", target_bir_lowering=False, debug=False)
    hs = nc.dram_tensor("hs", [S, Bc, H], mybir.dt.float32, kind="ExternalInput")
    wb = nc.dram_tensor("wb", [P, H], mybir.dt.float16, kind="ExternalInput")
    wbr = nc.dram_tensor(
        "wbr", [P, R4 * act_cols * H], mybir.dt.float16, kind="ExternalInput"
    )
    trilS = nc.dram_tensor("trilS", [P, P], mybir.dt.float32, kind="ExternalInput")
    triuI = nc.dram_tensor("triuI", [P, P], mybir.dt.float32, kind="ExternalInput")
    out = nc.dram_tensor("out", [S, Bc], mybir.dt.float32, kind="ExternalOutput")

    # partition p of supertile k holds rows t = 512k + 4p + i
    hs_r = hs.ap().rearrange("(blk p four) b h -> p blk four b h", p=P, four=R4)
    out_r = out.ap().rearrange("(blk p four) b -> p blk four b", p=P, four=R4)

    order = list(range(nsup - 1, -1, -1))  # reverse seq order
    # cols-per-DMA-call by processing index (ramp first, drain last)
    split_plan = {0: 2, 1: 4, nsup - 1: 2}

    with tile.TileContext(nc) as tc:
        with (
            tc.tile_pool(name="h16p", bufs=bufs16) as h16p,
            tc.tile_pool(name="consts", bufs=1) as consts,
            tc.tile_pool(name="work", bufs=1) as work,
            tc.tile_pool(name="sp", bufs=3) as sp,
            tc.tile_pool(name="ep", bufs=3) as ep,
            tc.tile_pool(name="tp", bufs=3) as tp,
            tc.tile_pool(name="dp", bufs=2) as dp,
            tc.tile_pool(name="prodp", bufs=2) as prodp,
            tc.tile_pool(name="psum", bufs=1, space="PSUM") as psum,
        ):
            h16_tiles = {}

            def issue_dma(idx):
                k = order[idx]
                h16 = h16p.tile([P, R4, Bc, H], mybir.dt.float16)
                qb = min(split_plan.get(idx, Bc), Bc)
                for q in range(0, Bc, qb):
                    nc.gpsimd.dma_start(
                        out=h16[:, :, q : q + qb, :].rearrange(
                            "p four b h -> p four (b h)"
                        ),
                        in_=hs_r[:, k, :, q : q + qb, :].rearrange(
                            "p four b h -> p four (b h)"
                        ),
                    )
                h16_tiles[k] = h16

            wb_t = consts.tile([P, H], mybir.dt.float16)
            nc.sync.dma_start(out=wb_t, in_=wb.ap())
            wbr_t = consts.tile([P, R4, act_cols, H], mybir.dt.float16)
            nc.sync.dma_start(
                out=wbr_t.rearrange("p four b h -> p (four b h)"), in_=wbr.ap()
            )
            trilS_t = consts.tile([P, P], mybir.dt.float32)
            nc.sync.dma_start(out=trilS_t, in_=trilS.ap())
            triuI_t = consts.tile([P, P], mybir.dt.float32)
            nc.sync.dma_start(out=triuI_t, in_=triuI.ap())

            for idx in range(look):
                issue_dma(idx)

            # Separate per-engine throwaway out-tiles: sharing one creates
            # false WAW dependencies that serialize the engines.
            dummy_v = work.tile([P, H], mybir.dt.float16)
            dummy_act = work.tile([P, H], mybir.dt.float16)
            sel_buf = work.tile([P, nsup * nw], mybir.dt.float32)
            r_ps = psum.tile([P, Bc], mybir.dt.float32)

            def emit_finalize(k, e_t, T_t):
                """Deferred: divisors, reciprocal, select, store; then R'."""
                # d_i = e_i + d_{i+1}; d_3 = e_3 + U (U is sitting in r_ps;
                # Pool cannot read PSUM, so this one add runs on DVE)
                d_t = dp.tile([P, nw], mybir.dt.float32)
                nc.vector.tensor_add(
                    d_t[:, 3 * Bc : 4 * Bc], e_t[:, 3 * Bc : 4 * Bc], r_ps
                )
                for i in (2, 1, 0):
                    nc.gpsimd.tensor_add(
                        d_t[:, i * Bc : (i + 1) * Bc],
                        e_t[:, i * Bc : (i + 1) * Bc],
                        d_t[:, (i + 1) * Bc : (i + 2) * Bc],
                    )
                rec_t = dp.tile([P, nw], mybir.dt.float32)
                nc.vector.reciprocal(rec_t, d_t)
                lo = k * nw
                nc.gpsimd.tensor_mul(sel_buf[:, lo : lo + nw], e_t, rec_t)
                nc.sync.dma_start(
                    out=out_r[:, k, :, :],
                    in_=sel_buf[:, lo : lo + nw].rearrange(
                        "p (four b) -> p four b", b=Bc
                    ),
                )
                # R' = U + sum_{p<=m} T = R + sum T, broadcast. Tile tracks
                # the WAR on r_ps (runs after the d_3 add's read of U).
                nc.tensor.matmul(r_ps, triuI_t, T_t, start=False, stop=True)

            pending = None  # (k, e_t, T_t) awaiting its deferred finalize
            for idx, k in enumerate(order):
                h16 = h16_tiles[k]
                s_t = sp.tile([P, nw], mybir.dt.float32)
                e_t = ep.tile([P, nw], mybir.dt.float32)
                T_t = tp.tile([P, Bc], mybir.dt.float32)

                # Products for the ACT-reduced (row, col) pairs: one 2x_1p
                # fp16 multiply covering all R4 rows x act_cols columns.
                if act_cols:
                    prod_t = prodp.tile([P, R4, act_cols, H], mybir.dt.float16)
                    eng = nc.vector if prod_on == "dve" else nc.gpsimd
                    eng.tensor_tensor(
                        prod_t,
                        h16[:, :, :act_cols, :],
                        wbr_t,
                        op=mybir.AluOpType.mult,
                    )
                    for i in range(R4):
                        for q in range(act_cols):
                            nc.scalar.activation(
                                dummy_act,
                                prod_t[:, i, q, :],
                                mybir.ActivationFunctionType.Copy,
                                accum_out=s_t[:, i * Bc + q : i * Bc + q + 1],
                            )
                for i in range(R4):
                    for b in range(act_cols, Bc):
                        nc.vector.scalar_tensor_tensor(
                            out=dummy_v,
                            in0=h16[:, i, b, :],
                            scalar=1.0,
                            in1=wb_t,
                            op0=mybir.AluOpType.mult,
                            op1=mybir.AluOpType.mult,
                            accum_out=s_t[:, i * Bc + b : i * Bc + b + 1],
                        )

                # Deferred finalize of the previous supertile: its U has been
                # sitting ready in PSUM, so nothing waits on the cross-engine
                # chain.
                if pending is not None:
                    pk, pe, pT = pending
                    emit_finalize(pk, pe, pT)

                if idx + look < nsup:
                    issue_dma(idx + look)

                nc.scalar.activation(e_t, s_t, mybir.ActivationFunctionType.Exp)
                # T[p] = sum_i e_i[p]  (3 small adds on Pool)
                t01 = tp.tile([P, Bc], mybir.dt.float32)
                t23 = tp.tile([P, Bc], mybir.dt.float32)
                nc.gpsimd.tensor_add(t01, e_t[:, 0:Bc], e_t[:, Bc : 2 * Bc])
                nc.gpsimd.tensor_add(
                    t23, e_t[:, 2 * Bc : 3 * Bc], e_t[:, 3 * Bc : 4 * Bc]
                )
                nc.gpsimd.tensor_add(T_t, t01, t23)
                # U[m] = R + sum_{p > m} T[p]
                nc.tensor.matmul(r_ps, trilS_t, T_t, start=(idx == 0), stop=True)
                pending = (k, e_t, T_t)

            pk, pe, pT = pending
            emit_finalize(pk, pe, pT)

    nc.compile()
    return nc


_PROGRAM = None


def _get_program():
    global _PROGRAM
    if _PROGRAM is None:
        _PROGRAM = build_program()
    return _PROGRAM


def make_in_maps(hidden_states, W, act_cols=4):
    hidden_states = np.asarray(hidden_states, dtype=np.float32)
    W = np.asarray(W, dtype=np.float32)
    wrow16 = np.broadcast_to(W[:, 0][None, :], (P, H)).astype(np.float16)
    wb = np.ascontiguousarray(wrow16)
    wbr = np.ascontiguousarray(np.tile(wrow16, (1, R4 * act_cols)))
    trilS = np.tril(np.ones((P, P), dtype=np.float32), -1)
    triuI = np.triu(np.ones((P, P), dtype=np.float32), 0)
    in_maps = []
    for c in range(N_CORES):
        hs_c = np.ascontiguousarray(hidden_states[:, c * BC : (c + 1) * BC, :])
        in_maps.append(
            {"hs": hs_c, "wb": wb, "wbr": wbr, "trilS": trilS, "triuI": triuI}
        )
    return in_maps


def assemble_output(results):
    cols = [results[c]["out"] for c in range(N_CORES)]
    return np.concatenate(cols, axis=1)[..., None].astype(np.float32)


def kernel(hidden_states, W, b):
    nc = _get_program()
    in_maps = make_in_maps(hidden_states, W)
    res = bass_utils.run_bass_kernel_spmd(nc, in_maps, core_ids=list(range(N_CORES)))
    return assemble_output(res.results)
